# revision 22
# baseline (speedup 1.0000x reference)
"""Trainium2 Bass kernel for nn_DiffusionNCA_fft2 (B=32, S=64, C=32, HID=256).

Self-contained: takes FULL inputs (as from setup_inputs()), shards batch over
8 NeuronCores (4 per core), runs one SPMD Bass program, gathers FULL output.

v2: deep software pipeline (2-batch front-end prefetch) to keep the PE warm,
strided-rhs matmuls for F2/IFFT-A so psum evacs are contiguous copies,
single-psum fc1 with full-width evacs, ACT restricted to Lrelu+Sqrt tables,
stats scalar chain on DVE, bounce DMAs spread across sync/scalar/gpsimd rings.
"""

import os
from contextlib import ExitStack

import numpy as np
import ml_dtypes

import concourse.bass as bass
import concourse.mybir as mybir
import concourse.tile as tile
from concourse import bacc

S = 64
C = 32
C2 = 64
C6 = 192
HID = 256
B = 32
NCORES = 8
BPC = B // NCORES            # batch per core
SP = 66                      # padded spatial
NPAD = SP * SP               # 4356
NPIX = S * S                 # 4096
LN_N = float(HID * NPIX)     # LN element count per batch
EPS = 1e-5
FIRE = 0.5

f32 = mybir.dt.float32
bf16 = mybir.dt.bfloat16
AF = mybir.ActivationFunctionType
ALU = mybir.AluOpType

_BF = ml_dtypes.bfloat16

STRIDED_RHS = bool(int(os.environ.get("KERNEL_STRIDED_RHS", "1")))


def _dft_mats():
    t = np.arange(S)
    ang = -2.0 * np.pi * np.outer(t, t) / S
    return np.cos(ang).astype(np.float32), np.sin(ang).astype(np.float32)


def host_constants(inp):
    """All per-core constant inputs, in device layouts (shared by all cores)."""
    Fr, Fi = _dft_mats()
    cst = {}

    ff1 = np.zeros((S, 2 * S), np.float32)
    ff1[:, :S], ff1[:, S:] = Fr.T, Fi.T
    cst["ff1"] = ff1.astype(_BF)

    w2 = np.zeros((2 * S, 2 * S), np.float32)
    w2[:S, :S], w2[S:, :S] = Fr.T, -Fi.T
    w2[:S, S:], w2[S:, S:] = Fi.T, Fr.T
    cst["w2"] = w2.astype(_BF)

    Gr, Gi = Fr / S, -Fi / S
    wa = np.zeros((2 * S, 2 * S), np.float32)
    wa[:S, :S], wa[S:, :S] = Gr.T, -Gi.T
    wa[:S, S:], wa[S:, S:] = Gi.T, Gr.T
    cst["wa"] = wa.astype(_BF)

    a = np.linspace(1.0, 0.0, S, dtype=np.float32)
    alive = (a[:, None] + a[None, :]) * 0.5
    cst["alive"] = np.pad(alive, 1, mode="reflect").reshape(-1).astype(_BF)
    cst["alive_int"] = alive.astype(_BF)         # unpadded [S, S]

    p0w, p1w = np.asarray(inp["p0_w"]), np.asarray(inp["p1_w"])
    wpair = np.zeros((2 * C2, 3 * 2 * C2), np.float32)  # [k, di*128 + m]
    wsing = np.zeros((C2, 3 * 2 * C2), np.float32)
    for di in range(3):
        mo = di * 2 * C2
        wpair[:C2, mo:mo + C2] = p0w[:, :, di, 0].T
        wpair[C2:, mo:mo + C2] = p0w[:, :, di, 1].T
        wpair[:C2, mo + C2:mo + 2 * C2] = p1w[:, :, di, 0].T
        wpair[C2:, mo + C2:mo + 2 * C2] = p1w[:, :, di, 1].T
        wsing[:, mo:mo + C2] = p0w[:, :, di, 2].T
        wsing[:, mo + C2:mo + 2 * C2] = p1w[:, :, di, 2].T
    cst["wpair"] = wpair.astype(_BF)
    cst["wsing"] = wsing.astype(_BF)

    ff2 = np.concatenate([ff1, ff1], axis=0)     # [128, 128] dup for row-tiling
    cst["ff2"] = ff2.astype(_BF)

    fc0w = np.asarray(inp["fc0_w"])
    fc0a2 = np.concatenate([fc0w[:C2], fc0w[:C2]], axis=0)  # [128, 256] dup
    cst["fc0a2"] = fc0a2.astype(_BF)
    cst["fc0bb"] = fc0w[C2:].astype(_BF)         # [128, 256]
    fc0b = (np.asarray(inp["fc0_b"])
            + np.asarray(inp["p0_b"]) @ fc0w[C2:2 * C2]
            + np.asarray(inp["p1_b"]) @ fc0w[2 * C2:])
    cst["fc0b2"] = fc0b.reshape(2, 128).T.astype(np.float32).copy()  # [128, 2]

    fc1w = np.asarray(inp["fc1_w"]).astype(np.float32)  # [256, 64]
    fc1t = np.zeros((128, 128), np.float32)
    fc1t[:, :64], fc1t[:, 64:] = fc1w[:128], fc1w[128:]
    cst["fc1"] = fc1t.astype(_BF)

    lnw = np.asarray(inp["ln_w"]).astype(np.float32)
    lnb = np.asarray(inp["ln_b"]).astype(np.float32)
    lnw_dev = np.transpose(lnw, (2, 1, 0)).reshape(HID, NPIX)  # [k, (a,b)]
    lnb_dev = np.transpose(lnb, (2, 1, 0)).reshape(HID, NPIX)
    cst["lnw"] = np.concatenate([lnw_dev[:128], lnw_dev[128:]], axis=1).astype(_BF)  # [128, 8192]
    lw1 = fc1w[:128].T @ lnw_dev[:128] + fc1w[128:].T @ lnw_dev[128:]  # [64, 4096]
    lb1 = fc1w[:128].T @ lnb_dev[:128] + fc1w[128:].T @ lnb_dev[128:]
    cst["lw1t"] = np.concatenate([lw1, lw1], axis=0).astype(_BF)  # [128, 4096] (2b dup)
    cst["lbt"] = np.concatenate([lb1, lb1], axis=0).astype(_BF)
    return cst


def build_nc(steps=1):
    nc = bacc.Bacc("TRN2", target_bir_lowering=False, debug=False)

    # ---- I/O ----
    xs = nc.dram_tensor("xs", [BPC, S, S, C], bf16, kind="ExternalInput")
    ins = {}
    cshape = dict(ff1=([S, 2 * S], bf16), ff2=([2 * S, 2 * S], bf16),
                  w2=([2 * S, 2 * S], bf16),
                  wa=([2 * S, 2 * S], bf16), alive=([NPAD], bf16),
                  alive_int=([S, S], bf16),
                  wpair=([2 * C2, 3 * 2 * C2], bf16), wsing=([C2, 3 * 2 * C2], bf16),
                  fc0a2=([2 * C2, HID], bf16), fc0bb=([2 * C2, HID], bf16),
                  fc0b2=([128, 2], f32), fc1=([128, 128], bf16),
                  lnw=([128, 2 * NPIX], bf16), lw1t=([128, NPIX], bf16),
                  lbt=([128, NPIX], bf16))
    for name, (shp, dt) in cshape.items():
        ins[name] = nc.dram_tensor(name, shp, dt, kind="ExternalInput")
    maskd = nc.dram_tensor("maskd", [BPC // 2, 128, NPIX], bf16, kind="ExternalInput")

    D1 = nc.dram_tensor("D1", [BPC, 2 * S, S * C], bf16)
    D2 = nc.dram_tensor("D2", [BPC, 2 * S, C * S], bf16)
    D3 = nc.dram_tensor("D3", [BPC // 2, 2, 2 * S, C * S], bf16)
    D4 = nc.dram_tensor("D4", [BPC, 2 * S, S * C], bf16)
    OUT = nc.dram_tensor("OUT", [BPC, 2 * S, S * C], bf16, kind="ExternalOutput")

    with tile.TileContext(nc) as tc, ExitStack() as ctx:
        cpool = ctx.enter_context(tc.tile_pool(name="consts", bufs=1))
        xpool = ctx.enter_context(tc.tile_pool(name="x", bufs=2))
        apool = ctx.enter_context(tc.tile_pool(name="stageA", bufs=2))
        bpool = ctx.enter_context(tc.tile_pool(name="stageB", bufs=2))
        s2pool = ctx.enter_context(tc.tile_pool(name="s2", bufs=2))
        dxpool = ctx.enter_context(tc.tile_pool(name="dx", bufs=2))
        ypool = ctx.enter_context(tc.tile_pool(name="yconv", bufs=2))
        hpool = ctx.enter_context(tc.tile_pool(name="h", bufs=4))
        scrpool = ctx.enter_context(tc.tile_pool(name="scr", bufs=1))
        spool = ctx.enter_context(tc.tile_pool(name="small", bufs=8))
        zpool = ctx.enter_context(tc.tile_pool(name="ztile", bufs=2))
        mpool = ctx.enter_context(tc.tile_pool(name="maskp", bufs=2))
        dmpool = ctx.enter_context(tc.tile_pool(name="dm", bufs=2))
        gpool = ctx.enter_context(tc.tile_pool(name="dgath", bufs=2))
        sapool = ctx.enter_context(tc.tile_pool(name="sa", bufs=2))
        gbpool = ctx.enter_context(tc.tile_pool(name="dgb", bufs=2))
        sbpool = ctx.enter_context(tc.tile_pool(name="sb", bufs=1))
        pmm = ctx.enter_context(tc.tile_pool(name="pmm", bufs=4, space="PSUM"))
        pfft = pmm

        # ---- constants to SBUF ----
        ct = {}
        for name, (shp, dt) in cshape.items():
            if name in ("alive", "alive_int"):
                continue
            t = cpool.tile(shp, dt, tag="c_" + name)
            nc.sync.dma_start(t[:], ins[name][:])
            ct[name] = t

        ones = cpool.tile([128, 128], f32, tag="c_ones")
        nc.gpsimd.memset(ones[:], 1.0)

        # seed the alive channel (ri=1, c=31) into every D2 slot once; the
        # per-batch D2 write skips those columns so this persists.
        for b in range(BPC):
            nc.gpsimd.dma_start(D2[b][S:2 * S, (C - 1) * S:C * S],
                                ins["alive_int"][:])

        # ---- cross-stage state ----
        h_tiles = {}
        stats = {}
        dgath = {}
        s1cols = {}
        s2cols = {}

        def fft_front(b):
            """X load -> F1 (row-tiled pairs) -> D1 bounce -> F2 -> s2 -> D2."""
            X = xpool.tile([2 * S, S * C], bf16, tag="X", name=f"X_{b}")
            nc.sync.dma_start(X[0:S, :], xs[b].rearrange("a b c -> a (b c)"))
            nc.scalar.dma_start(X[S:2 * S, :], xs[b].rearrange("a b c -> a (b c)"))
            t1d = apool.tile([2 * S, S * C], bf16, tag="stageA", name=f"t1d_{b}")
            for half in range(2):
                ps = pfft.tile([2 * S, 1024], f32, tag="pmm")
                nc.tensor.matmul(ps[:, bass.ts(0, 512)], ct["ff2"][0:S, :],
                                 X[0:S, bass.ts(half * 2, 512)],
                                 tile_position=(0, 0))
                nc.tensor.matmul(ps[:, bass.ts(1, 512)], ct["ff2"][S:2 * S, :],
                                 X[S:2 * S, bass.ts(half * 2 + 1, 512)],
                                 tile_position=(64, 0))
                nc.vector.tensor_copy(t1d[:, bass.ts(half, 1024)], ps[:])
            nc.sync.dma_start(D1[b][:], t1d[:])
            # bounce 1 -> t1g [(ri s1), (v c)]
            t1g = bpool.tile([2 * S, S * C], bf16, tag="stageB", name=f"t1g_{b}")
            d1v = D1[b].rearrange("(ri v) (s1 c) -> ri s1 v c", ri=2, v=S, s1=S, c=C)
            nc.sync.dma_start(
                t1g[bass.ts(0, S), :].rearrange("p (v c) -> p v c", v=S, c=C), d1v[0])
            nc.scalar.dma_start(
                t1g[bass.ts(1, S), :].rearrange("p (v c) -> p v c", v=S, c=C), d1v[1])
            # F2 -> s2 [(ri fb), (c v)]
            s2 = s2pool.tile([2 * S, C * S], bf16, tag="s2", name=f"s2_{b}")
            if STRIDED_RHS:
                t1gv = t1g[:, :].rearrange("p (v c) -> p c v", v=S, c=C)
                for half in range(2):
                    ps = pfft.tile([2 * S, 1024], f32, tag="pmm")
                    for q in range(2):
                        cq = half * 2 + q
                        nc.tensor.matmul(ps[:, bass.ts(q, 512)], ct["w2"][:],
                                         t1gv[:, bass.ts(cq, 8), :])
                    nc.vector.tensor_copy(s2[:, bass.ts(half, 1024)], ps[:])
            else:
                for half in range(2):
                    ps = pfft.tile([2 * S, 1024], f32, tag="pmm")
                    for q in range(2):
                        nc.tensor.matmul(ps[:, bass.ts(q, 512)], ct["w2"][:],
                                         t1g[:, bass.ds(half * 1024 + q * 512, 512)])
                    nc.vector.tensor_copy(
                        s2[:].rearrange("p (c v) -> p v c", c=C, v=S)[:, bass.ts(half, 32), :],
                        ps[:].rearrange("p (v c) -> p v c", v=32, c=C))
            nc.sync.dma_start(D2[b][0:S, :], s2[0:S, :])
            nc.scalar.dma_start(D2[b][S:2 * S, 0:(C - 1) * S],
                                s2[S:2 * S, 0:(C - 1) * S])

        def build_dx(b):
            """D2 reads into padded dx layout + reflect pads, split in row
            chunks so conv T0/T1 can start before the full transfer lands."""
            dx2 = dxpool.tile([2 * C2, NPAD], bf16, tag="dx2", name=f"dx2_{b}")
            dxv = dx2[:, 0:NPAD].rearrange("q (a b) -> q a b", a=SP, b=SP)
            d2v = D2[b].rearrange("(ri u) (c v) -> ri c u v", ri=2, u=S, c=C, v=S)
            # row chunks (interior row ranges): [1,34) and [34,65)
            RCH = ((1, 34), (34, S + 1))
            for (lo, hi) in RCH:
                nc.sync.dma_start(dxv[0:32, lo:hi, 1:S + 1], d2v[0][:, lo - 1:hi - 1])
                nc.scalar.dma_start(dxv[32:64, lo:hi, 1:S + 1], d2v[1][:, lo - 1:hi - 1])
                nc.gpsimd.dma_start(dxv[64:96, lo:hi, 0:S], d2v[0][:, lo - 1:hi - 1])
                nc.gpsimd.dma_start(dxv[96:128, lo:hi, 0:S], d2v[1][:, lo - 1:hi - 1])
                q = slice(0, C2)
                nc.vector.tensor_copy(dxv[q, lo:hi, 0:1], dxv[q, lo:hi, 2:3])
                nc.vector.tensor_copy(dxv[q, lo:hi, SP - 1:SP],
                                      dxv[q, lo:hi, SP - 3:SP - 2])
            q = slice(0, C2)
            nc.vector.tensor_copy(dxv[q, 0:1, :], dxv[q, 2:3, :])
            nc.vector.tensor_copy(dxv[q, SP - 1:SP, :], dxv[q, SP - 3:SP - 2, :])
            qb = slice(C2, 2 * C2)
            nc.vector.tensor_copy(dxv[qb, 0:1, 0:S], dxv[qb, 2:3, 0:S])
            nc.vector.tensor_copy(dxv[qb, SP - 1:SP, 0:S],
                                  dxv[qb, SP - 3:SP - 2, 0:S])
            return dx2

        def conv_fc0(b, dx2):
            """3x3 convs + fc0 + LeakyReLU + LN partial sums."""
            dxv = dx2[:, 0:NPAD].rearrange("q (a b) -> q a b", a=SP, b=SP)
            s1c = spool.tile([128, 8], f32, tag="s1cols", name=f"s1c_{b}")
            s2c = spool.tile([128, 4], f32, tag="s2cols", name=f"s2c_{b}")
            s1cols[b], s2cols[b] = s1c, s2c
            for m in range(2):
                h_tiles[(b, m)] = hpool.tile([128, NPIX], bf16, tag="h",
                                             name=f"h_{b}_{m}")
            for T in range(4):
                r0 = T * 16
                psy = pmm.tile([2 * C2, 1024], f32, tag="pmm")
                for q in range(2):
                    rq = r0 + q * 8
                    for di in range(3):
                        nc.tensor.matmul(
                            psy[:, bass.ts(q, 512)],
                            ct["wpair"][:, bass.ts(di, 2 * C2)],
                            dxv[:, rq + di:rq + di + 8, 0:S],
                            start=(di == 0), stop=False)
                    for di in range(2):
                        nc.tensor.matmul(
                            psy[:, bass.ts(q, 512)],
                            ct["wsing"][:, bass.ts(di, 2 * C2)],
                            dxv[0:C2, rq + di:rq + di + 8, 2:SP],
                            start=False, stop=False)
                psh = {m: pmm.tile([128, 1024], f32, tag="pmm",
                                   name=f"psh_{b}_{T}_{m}") for m in range(2)}
                # row-tiled concurrent pairs: single(di=2)@rows0:63 with
                # fc0a-m0@rows64:127 (B-half rhs), then fc0a-m1 q0||q1.
                for q in range(2):
                    rq = r0 + q * 8
                    nc.tensor.matmul(
                        psy[:, bass.ts(q, 512)],
                        ct["wsing"][:, bass.ts(2, 2 * C2)],
                        dxv[0:C2, rq + 2:rq + 10, 2:SP],
                        start=False, stop=True, tile_position=(0, 0))
                    nc.tensor.matmul(
                        psh[0][:, bass.ts(q, 512)],
                        ct["fc0a2"][C2:2 * C2, bass.ts(0, 128)],
                        dxv[C2:2 * C2, rq + 1:rq + 9, 0:S],
                        start=True, stop=False, tile_position=(64, 0))
                nc.tensor.matmul(psh[1][:, bass.ts(0, 512)],
                                 ct["fc0a2"][0:C2, bass.ts(1, 128)],
                                 dxv[0:C2, r0 + 1:r0 + 9, 1:S + 1],
                                 start=True, stop=False, tile_position=(0, 0))
                nc.tensor.matmul(psh[1][:, bass.ts(1, 512)],
                                 ct["fc0a2"][C2:2 * C2, bass.ts(1, 128)],
                                 dxv[C2:2 * C2, r0 + 9:r0 + 17, 0:S],
                                 start=True, stop=False, tile_position=(64, 0))
                yc = ypool.tile([2 * C2, 1024], bf16, tag="yc")
                nc.vector.tensor_copy(yc[:], psy[:])
                for m in range(2):
                    for q in range(2):
                        nc.tensor.matmul(psh[m][:, bass.ts(q, 512)],
                                         ct["fc0bb"][:, bass.ts(m, 128)],
                                         yc[:, bass.ts(q, 512)],
                                         start=False, stop=True)
                    nc.scalar.activation(
                        h_tiles[(b, m)][:, bass.ts(T, 1024)], psh[m][:],
                        AF.Lrelu, bias=ct["fc0b2"][:, m:m + 1], scale=1.0,
                        alpha=0.01, accum_out=s1c[:, T * 2 + m:T * 2 + m + 1])

        def stats_pre(b):
            """sq-pass (TT + reduce) -> per-partition sums -> ones-matmul,
            plus the hw-pass (h *= ln_w), all independent of the pair."""
            scr = scrpool.tile([128, 2048], bf16, tag="sqscr")
            for m in range(2):
                for t in range(2):
                    hs = h_tiles[(b, m)][:, bass.ts(t, 2048)]
                    nc.vector.tensor_tensor(out=scr[:], in0=hs, in1=hs,
                                            op=ALU.mult)
                    nc.vector.tensor_reduce(
                        s2cols[b][:, m * 2 + t:m * 2 + t + 1], scr[:],
                        axis=mybir.AxisListType.X, op=ALU.add)
            stats2 = spool.tile([128, 2], f32, tag="stats2", name=f"stats2_{b}")
            nc.vector.tensor_reduce(stats2[:, 0:1], s1cols[b][:],
                                    axis=mybir.AxisListType.X, op=ALU.add)
            nc.vector.tensor_reduce(stats2[:, 1:2], s2cols[b][:],
                                    axis=mybir.AxisListType.X, op=ALU.add)
            pst = pmm.tile([128, 2], f32, tag="pmm", name=f"pst_{b}")
            nc.tensor.matmul(pst[:], ones[:], stats2[:])
            psts = spool.tile([128, 2], f32, tag="psts", name=f"psts_{b}")
            nc.vector.tensor_copy(psts[:], pst[:])
            stats[b] = {"pst": psts}
            for m in range(2):
                nc.vector.tensor_mul(h_tiles[(b, m)][:], h_tiles[(b, m)][:],
                                     ct["lnw"][:, bass.ts(m, NPIX)])

        def stats_pair(pair):
            """Merged scalar chain for both batches: one Sqrt instruction."""
            b0, b1 = 2 * pair, 2 * pair + 1
            mu2 = spool.tile([128, 2], f32, tag="stat2", name=f"mu2_{pair}")
            ex2 = spool.tile([128, 2], f32, tag="stat2", name=f"ex2_{pair}")
            for j, bb in ((0, b0), (1, b1)):
                nc.vector.tensor_scalar(out=mu2[:, j:j + 1],
                                        in0=stats[bb]["pst"][:, 0:1],
                                        scalar1=1.0 / LN_N, scalar2=None,
                                        op0=ALU.mult)
                nc.vector.tensor_scalar(out=ex2[:, j:j + 1],
                                        in0=stats[bb]["pst"][:, 1:2],
                                        scalar1=1.0 / LN_N, scalar2=None,
                                        op0=ALU.mult)
            msq = spool.tile([128, 2], f32, tag="stat2", name=f"msq_{pair}")
            nc.vector.tensor_tensor(out=msq[:], in0=mu2[:], in1=mu2[:],
                                    op=ALU.mult)
            vpe = spool.tile([128, 2], f32, tag="stat2", name=f"vpe_{pair}")
            nc.vector.tensor_tensor(out=vpe[:], in0=ex2[:], in1=msq[:],
                                    op=ALU.subtract)
            nc.vector.tensor_scalar_add(vpe[:], vpe[:], EPS)
            sd = spool.tile([128, 2], f32, tag="stat2", name=f"sd_{pair}")
            nc.scalar.activation(sd[:], vpe[:], AF.Sqrt, bias=0.0, scale=1.0)
            rr = spool.tile([128, 2], f32, tag="stat2", name=f"rr_{pair}")
            nc.vector.reciprocal(rr[:], sd[:])
            nrm = spool.tile([128, 2], f32, tag="stat2", name=f"nrm_{pair}")
            nc.vector.tensor_tensor(out=nrm[:], in0=rr[:], in1=mu2[:],
                                    op=ALU.mult)
            nc.vector.tensor_scalar_mul(nrm[:], nrm[:], -1.0)
            # pack per-batch halves: rows 0:64 <- b0 col, rows 64:128 <- b1 col
            r2 = spool.tile([128, 1], f32, tag="statp", name=f"r2_{pair}")
            nrm2 = spool.tile([128, 1], f32, tag="statp", name=f"nrm2_{pair}")
            nc.vector.tensor_copy(r2[0:64, :], rr[0:64, 0:1])
            nc.vector.tensor_copy(r2[64:128, :], rr[64:128, 1:2])
            nc.vector.tensor_copy(nrm2[0:64, :], nrm[0:64, 0:1])
            nc.vector.tensor_copy(nrm2[64:128, :], nrm[64:128, 1:2])
            return r2, nrm2

        def fc1_tail(pair):
            b0, b1 = 2 * pair, 2 * pair + 1
            r2, nrm2 = stats_pair(pair)
            z = zpool.tile([128, NPIX], bf16, tag="ztile")
            nc.vector.scalar_tensor_tensor(
                out=z[:], in0=ct["lw1t"][:], scalar=nrm2[:], in1=ct["lbt"][:],
                op0=ALU.mult, op1=ALU.add)
            mask2 = mpool.tile([128, NPIX], bf16, tag="mask2", name=f"mask2_{pair}")
            nc.sync.dma_start(mask2[:], maskd[pair][:])
            dm = dmpool.tile([128, NPIX], bf16, tag="dm")
            for T in range(4):
                ps = pmm.tile([128, 1024], f32, tag="pmm", name=f"psd_{pair}_{T}")
                for q in range(2):
                    for m in range(2):
                        for half, b in ((0, b0), (1, b1)):
                            nc.tensor.matmul(
                                ps[bass.ts(half, 64), bass.ts(q, 512)],
                                ct["fc1"][:, bass.ts(m, 64)],
                                h_tiles[(b, m)][:, bass.ds(T * 1024 + q * 512, 512)],
                                start=(m == 0), stop=(m == 1),
                                tile_position=(0, half * 64))
                nc.vector.scalar_tensor_tensor(
                    out=dm[:, bass.ts(T, 1024)], in0=ps[:],
                    scalar=r2[:], in1=z[:, bass.ts(T, 1024)],
                    op0=ALU.mult, op1=ALU.add)
            nc.vector.tensor_mul(dm[:], dm[:], mask2[:])
            for hb, b in ((0, b0), (1, b1)):
                eng = nc.sync if hb == 0 else nc.scalar
                for ri in range(2):
                    nc.sync.dma_start(
                        D3[pair][hb].rearrange("(ri u) (c v) -> ri c u v",
                                               ri=2, u=S, c=C, v=S)[ri],
                        dm[bass.ds(hb * 64 + ri * 32, 32), :].rearrange(
                            "c (u v) -> c u v", u=S, v=S))
                dg = gpool.tile([2 * S, C * S], bf16, tag="dg", name=f"dg_{b}")
                d3g = D3[pair][hb].rearrange("(ri u) (c v) -> ri u c v",
                                             ri=2, u=S, c=C, v=S)
                for ri in range(2):
                    eng.dma_start(
                        dg[bass.ts(ri, S), :].rearrange("p (c v) -> p c v", c=C, v=S),
                        d3g[ri])
                dgath[b] = dg

        def ifft_out(b):
            upd = dgath[b]
            sa = sapool.tile([2 * S, S * C], bf16, tag="sa", name=f"sa_{b}")
            if STRIDED_RHS:
                updv = upd[:, :].rearrange("p (c v) -> p v c", c=C, v=S)
                for half in range(2):
                    ps = pfft.tile([2 * S, 1024], f32, tag="pmm")
                    for q in range(2):
                        vq = half * 2 + q
                        nc.tensor.matmul(ps[:, bass.ts(q, 512)], ct["wa"][:],
                                         updv[:, bass.ts(vq, 16), :])
                    nc.vector.tensor_copy(sa[:, bass.ts(half, 1024)], ps[:])
            else:
                for half in range(2):
                    ps = pfft.tile([2 * S, 1024], f32, tag="pmm")
                    for q in range(2):
                        nc.tensor.matmul(ps[:, bass.ts(q, 512)], ct["wa"][:],
                                         upd[:, bass.ds(half * 1024 + q * 512, 512)])
                    nc.vector.tensor_copy(
                        sa[:].rearrange("p (v c) -> p c v", v=S, c=C)[:, bass.ts(half, 16), :],
                        ps[:].rearrange("p (c v) -> p c v", c=16, v=S))
            nc.sync.dma_start(D4[b][:], sa[:])
            dgb = gbpool.tile([2 * S, S * C], bf16, tag="dgb", name=f"dgb_{b}")
            d4v = D4[b].rearrange("(ri a) (v c) -> ri v a c", ri=2, a=S, v=S, c=C)
            nc.sync.dma_start(
                dgb[bass.ts(0, S), :].rearrange("p (a c) -> p a c", a=S, c=C), d4v[0])
            nc.scalar.dma_start(
                dgb[bass.ts(1, S), :].rearrange("p (a c) -> p a c", a=S, c=C), d4v[1])
            sb = sbpool.tile([2 * S, S * C], bf16, tag="sb", name=f"sb_{b}")
            for half in range(2):
                ps = pfft.tile([2 * S, 1024], f32, tag="pmm")
                for q in range(2):
                    nc.tensor.matmul(ps[:, bass.ts(q, 512)], ct["wa"][:],
                                     dgb[:, bass.ds(half * 1024 + q * 512, 512)])
                nc.vector.tensor_copy(sb[:, bass.ts(half, 1024)], ps[:])
            nc.sync.dma_start(OUT[b][:], sb[:])

        assert steps == 1, "device program built for steps==1"
        # ---- software-pipelined emission, 2-deep front-end prefetch ----
        # fronts: 0,1 up-front; dx(b) as early as deps allow; tails late.
        fft_front(0)
        fft_front(1)
        dx0 = build_dx(0)
        fft_front(2)
        dx1 = build_dx(1)
        conv_fc0(0, dx0)
        fft_front(3)
        stats_pre(0)
        conv_fc0(1, dx1)
        stats_pre(1)
        dx2_ = build_dx(2)
        fc1_tail(0)
        ifft_out(0)
        conv_fc0(2, dx2_)
        stats_pre(2)
        dx3 = build_dx(3)
        ifft_out(1)
        conv_fc0(3, dx3)
        stats_pre(3)
        fc1_tail(1)
        ifft_out(2)
        ifft_out(3)

    return nc


_BUILT = {}


def kernel(**inputs):
    x = np.ascontiguousarray(np.asarray(inputs["x"], dtype=np.float32))
    steps = int(np.asarray(inputs["steps"]))
    if steps == 0:
        return x.astype(np.complex64)
    assert steps == 1, f"unsupported steps={steps}"

    cst = host_constants(inputs)
    su = np.asarray(inputs["stoch_u"], dtype=np.float32)[..., 0]   # [B, S, S]
    mask = (su > FIRE).astype(np.float32)
    mask_dev = np.ascontiguousarray(np.transpose(mask, (0, 2, 1))
                                    ).reshape(B, NPIX).astype(_BF)
    mask_pairs = np.empty((B // 2, 128, NPIX), _BF)
    for p in range(B // 2):
        mask_pairs[p, :64] = mask_dev[2 * p][None, :]
        mask_pairs[p, 64:] = mask_dev[2 * p + 1][None, :]

    if "nc" not in _BUILT:
        nc = build_nc(steps=1)
        nc.finalize()
        _BUILT["nc"] = nc
    nc = _BUILT["nc"]

    in_maps = []
    for core in range(NCORES):
        m = {k: np.ascontiguousarray(v) for k, v in cst.items()}
        m["xs"] = x[core * BPC:(core + 1) * BPC].astype(_BF)
        m["maskd"] = mask_pairs[core * (BPC // 2):(core + 1) * (BPC // 2)]
        in_maps.append(m)

    from concourse.bass_utils import run_bass_kernel_spmd
    trace = bool(int(os.environ.get("KERNEL_TRACE", "0")))
    res = run_bass_kernel_spmd(nc, in_maps, list(range(NCORES)), trace=trace)
    if trace and res.exec_time_ns is not None:
        print(f"HW exec time: {res.exec_time_ns} ns")
        if res.instructions_and_trace is not None:
            print("trace:", res.instructions_and_trace[1])

    out = np.empty((B, S, S, C), np.complex64)
    for core in range(NCORES):
        o = np.asarray(res.results[core]["OUT"], dtype=np.float32)  # [BPC,128,2048]
        for j in range(BPC):
            b = core * BPC + j
            re = o[j, :S].reshape(S, S, C)
            im = o[j, S:].reshape(S, S, C)
            out[b] = x[b] + re + 1j * im
    return out


# revision 23
# speedup vs baseline: 1.0490x; 1.0490x over previous
"""Trainium2 Bass kernel for nn_DiffusionNCA_fft2 (B=32, S=64, C=32, HID=256).

Self-contained: takes FULL inputs (as from setup_inputs()), shards batch over
8 NeuronCores (4 per core), runs one SPMD Bass program, gathers FULL output.

v2: deep software pipeline (2-batch front-end prefetch) to keep the PE warm,
strided-rhs matmuls for F2/IFFT-A so psum evacs are contiguous copies,
single-psum fc1 with full-width evacs, ACT restricted to Lrelu+Sqrt tables,
stats scalar chain on DVE, bounce DMAs spread across sync/scalar/gpsimd rings.
"""

import os
from contextlib import ExitStack

import numpy as np
import ml_dtypes

import concourse.bass as bass
import concourse.mybir as mybir
import concourse.tile as tile
from concourse import bacc

S = 64
C = 32
C2 = 64
C6 = 192
HID = 256
B = 32
NCORES = 8
BPC = B // NCORES            # batch per core
SP = 66                      # padded spatial
NPAD = SP * SP               # 4356
NPIX = S * S                 # 4096
LN_N = float(HID * NPIX)     # LN element count per batch
EPS = 1e-5
FIRE = 0.5

f32 = mybir.dt.float32
bf16 = mybir.dt.bfloat16
AF = mybir.ActivationFunctionType
ALU = mybir.AluOpType

_BF = ml_dtypes.bfloat16

STRIDED_RHS = bool(int(os.environ.get("KERNEL_STRIDED_RHS", "1")))


def _dft_mats():
    t = np.arange(S)
    ang = -2.0 * np.pi * np.outer(t, t) / S
    return np.cos(ang).astype(np.float32), np.sin(ang).astype(np.float32)


def host_constants(inp):
    """All per-core constant inputs, in device layouts (shared by all cores)."""
    Fr, Fi = _dft_mats()
    cst = {}

    ff1 = np.zeros((S, 2 * S), np.float32)
    ff1[:, :S], ff1[:, S:] = Fr.T, Fi.T
    cst["ff1"] = ff1.astype(_BF)

    w2 = np.zeros((2 * S, 2 * S), np.float32)
    w2[:S, :S], w2[S:, :S] = Fr.T, -Fi.T
    w2[:S, S:], w2[S:, S:] = Fi.T, Fr.T
    cst["w2"] = w2.astype(_BF)

    Gr, Gi = Fr / S, -Fi / S
    wa = np.zeros((2 * S, 2 * S), np.float32)
    wa[:S, :S], wa[S:, :S] = Gr.T, -Gi.T
    wa[:S, S:], wa[S:, S:] = Gi.T, Gr.T
    cst["wa"] = wa.astype(_BF)

    a = np.linspace(1.0, 0.0, S, dtype=np.float32)
    alive = (a[:, None] + a[None, :]) * 0.5
    cst["alive"] = np.pad(alive, 1, mode="reflect").reshape(-1).astype(_BF)
    cst["alive_int"] = alive.astype(_BF)         # unpadded [S, S]

    p0w, p1w = np.asarray(inp["p0_w"]), np.asarray(inp["p1_w"])
    wpair = np.zeros((2 * C2, 3 * 2 * C2), np.float32)  # [k, di*128 + m]
    wsing = np.zeros((C2, 3 * 2 * C2), np.float32)
    for di in range(3):
        mo = di * 2 * C2
        wpair[:C2, mo:mo + C2] = p0w[:, :, di, 0].T
        wpair[C2:, mo:mo + C2] = p0w[:, :, di, 1].T
        wpair[:C2, mo + C2:mo + 2 * C2] = p1w[:, :, di, 0].T
        wpair[C2:, mo + C2:mo + 2 * C2] = p1w[:, :, di, 1].T
        wsing[:, mo:mo + C2] = p0w[:, :, di, 2].T
        wsing[:, mo + C2:mo + 2 * C2] = p1w[:, :, di, 2].T
    cst["wpair"] = wpair.astype(_BF)
    cst["wsing"] = wsing.astype(_BF)

    ff2 = np.concatenate([ff1, ff1], axis=0)     # [128, 128] dup for row-tiling
    cst["ff2"] = ff2.astype(_BF)

    fc0w = np.asarray(inp["fc0_w"])
    fc0a2 = np.concatenate([fc0w[:C2], fc0w[:C2]], axis=0)  # [128, 256] dup
    cst["fc0a2"] = fc0a2.astype(_BF)
    cst["fc0bb"] = fc0w[C2:].astype(_BF)         # [128, 256]
    fc0b = (np.asarray(inp["fc0_b"])
            + np.asarray(inp["p0_b"]) @ fc0w[C2:2 * C2]
            + np.asarray(inp["p1_b"]) @ fc0w[2 * C2:])
    cst["fc0b2"] = fc0b.reshape(2, 128).T.astype(np.float32).copy()  # [128, 2]

    fc1w = np.asarray(inp["fc1_w"]).astype(np.float32)  # [256, 64]
    fc1t = np.zeros((128, 128), np.float32)
    fc1t[:, :64], fc1t[:, 64:] = fc1w[:128], fc1w[128:]
    cst["fc1"] = fc1t.astype(_BF)

    lnw = np.asarray(inp["ln_w"]).astype(np.float32)
    lnb = np.asarray(inp["ln_b"]).astype(np.float32)
    lnw_dev = np.transpose(lnw, (2, 1, 0)).reshape(HID, NPIX)  # [k, (a,b)]
    lnb_dev = np.transpose(lnb, (2, 1, 0)).reshape(HID, NPIX)
    cst["lnw"] = np.concatenate([lnw_dev[:128], lnw_dev[128:]], axis=1).astype(_BF)  # [128, 8192]
    lw1 = fc1w[:128].T @ lnw_dev[:128] + fc1w[128:].T @ lnw_dev[128:]  # [64, 4096]
    lb1 = fc1w[:128].T @ lnb_dev[:128] + fc1w[128:].T @ lnb_dev[128:]
    cst["lw1t"] = np.concatenate([lw1, lw1], axis=0).astype(_BF)  # [128, 4096] (2b dup)
    cst["lbt"] = np.concatenate([lb1, lb1], axis=0).astype(_BF)
    return cst


def build_nc(steps=1):
    nc = bacc.Bacc("TRN2", target_bir_lowering=False, debug=False)

    # ---- I/O ----
    xs = nc.dram_tensor("xs", [BPC, S, S, C], bf16, kind="ExternalInput")
    ins = {}
    cshape = dict(ff1=([S, 2 * S], bf16), ff2=([2 * S, 2 * S], bf16),
                  w2=([2 * S, 2 * S], bf16),
                  wa=([2 * S, 2 * S], bf16), alive=([NPAD], bf16),
                  alive_int=([S, S], bf16),
                  wpair=([2 * C2, 3 * 2 * C2], bf16), wsing=([C2, 3 * 2 * C2], bf16),
                  fc0a2=([2 * C2, HID], bf16), fc0bb=([2 * C2, HID], bf16),
                  fc0b2=([128, 2], f32), fc1=([128, 128], bf16),
                  lnw=([128, 2 * NPIX], bf16), lw1t=([128, NPIX], bf16),
                  lbt=([128, NPIX], bf16))
    for name, (shp, dt) in cshape.items():
        ins[name] = nc.dram_tensor(name, shp, dt, kind="ExternalInput")
    maskd = nc.dram_tensor("maskd", [BPC // 2, 128, NPIX], bf16, kind="ExternalInput")

    D1 = nc.dram_tensor("D1", [BPC, 2 * S, S * C], bf16)
    D2 = nc.dram_tensor("D2", [BPC, 2 * S, C * S], bf16)
    D3 = nc.dram_tensor("D3", [BPC // 2, 2, 2 * S, C * S], bf16)
    D4 = nc.dram_tensor("D4", [BPC, 2 * S, S * C], bf16)
    OUT = nc.dram_tensor("OUT", [BPC, 2 * S, S * C], bf16, kind="ExternalOutput")

    with tile.TileContext(nc) as tc, ExitStack() as ctx:
        cpool = ctx.enter_context(tc.tile_pool(name="consts", bufs=1))
        xpool = ctx.enter_context(tc.tile_pool(name="x", bufs=2))
        apool = ctx.enter_context(tc.tile_pool(name="stageA", bufs=2))
        bpool = ctx.enter_context(tc.tile_pool(name="stageB", bufs=2))
        s2pool = ctx.enter_context(tc.tile_pool(name="s2", bufs=2))
        dxpool = ctx.enter_context(tc.tile_pool(name="dx", bufs=3))
        ypool = ctx.enter_context(tc.tile_pool(name="yconv", bufs=2))
        hpool = ctx.enter_context(tc.tile_pool(name="h", bufs=4))
        scrpool = ctx.enter_context(tc.tile_pool(name="scr", bufs=1))
        spool = ctx.enter_context(tc.tile_pool(name="small", bufs=8))
        zpool = ctx.enter_context(tc.tile_pool(name="ztile", bufs=2))
        mpool = ctx.enter_context(tc.tile_pool(name="maskp", bufs=2))
        dmpool = ctx.enter_context(tc.tile_pool(name="dm", bufs=2))
        gpool = ctx.enter_context(tc.tile_pool(name="dgath", bufs=2))
        sapool = ctx.enter_context(tc.tile_pool(name="sa", bufs=1))
        gbpool = ctx.enter_context(tc.tile_pool(name="dgb", bufs=2))
        sbpool = ctx.enter_context(tc.tile_pool(name="sb", bufs=1))
        pmm = ctx.enter_context(tc.tile_pool(name="pmm", bufs=4, space="PSUM"))
        pfft = pmm

        # ---- constants to SBUF ----
        ct = {}
        for name, (shp, dt) in cshape.items():
            if name in ("alive", "alive_int"):
                continue
            t = cpool.tile(shp, dt, tag="c_" + name)
            nc.sync.dma_start(t[:], ins[name][:])
            ct[name] = t

        ones = cpool.tile([128, 128], f32, tag="c_ones")
        nc.gpsimd.memset(ones[:], 1.0)

        # seed the alive channel (ri=1, c=31) into every D2 slot once; the
        # per-batch D2 write skips those columns so this persists.
        for b in range(BPC):
            nc.gpsimd.dma_start(D2[b][S:2 * S, (C - 1) * S:C * S],
                                ins["alive_int"][:])

        # ---- cross-stage state ----
        h_tiles = {}
        stats = {}
        dgath = {}
        s1cols = {}
        s2cols = {}

        def fft_front(b):
            """X load -> F1 (row-tiled pairs) -> D1 bounce -> F2 -> s2 -> D2."""
            X = xpool.tile([2 * S, S * C], bf16, tag="X", name=f"X_{b}")
            nc.sync.dma_start(X[0:S, :], xs[b].rearrange("a b c -> a (b c)"))
            nc.scalar.dma_start(X[S:2 * S, :], xs[b].rearrange("a b c -> a (b c)"))
            t1d = apool.tile([2 * S, S * C], bf16, tag="stageA", name=f"t1d_{b}")
            for half in range(2):
                ps = pfft.tile([2 * S, 1024], f32, tag="pmm")
                nc.tensor.matmul(ps[:, bass.ts(0, 512)], ct["ff2"][0:S, :],
                                 X[0:S, bass.ts(half * 2, 512)],
                                 tile_position=(0, 0))
                nc.tensor.matmul(ps[:, bass.ts(1, 512)], ct["ff2"][S:2 * S, :],
                                 X[S:2 * S, bass.ts(half * 2 + 1, 512)],
                                 tile_position=(64, 0))
                nc.vector.tensor_copy(t1d[:, bass.ts(half, 1024)], ps[:])
            nc.sync.dma_start(D1[b][:], t1d[:])
            # bounce 1 -> t1g [(ri s1), (v c)]
            t1g = bpool.tile([2 * S, S * C], bf16, tag="stageB", name=f"t1g_{b}")
            d1v = D1[b].rearrange("(ri v) (s1 c) -> ri s1 v c", ri=2, v=S, s1=S, c=C)
            nc.sync.dma_start(
                t1g[bass.ts(0, S), :].rearrange("p (v c) -> p v c", v=S, c=C), d1v[0])
            nc.scalar.dma_start(
                t1g[bass.ts(1, S), :].rearrange("p (v c) -> p v c", v=S, c=C), d1v[1])
            # F2 -> s2 [(ri fb), (c v)]
            s2 = s2pool.tile([2 * S, C * S], bf16, tag="s2", name=f"s2_{b}")
            if STRIDED_RHS:
                t1gv = t1g[:, :].rearrange("p (v c) -> p c v", v=S, c=C)
                for half in range(2):
                    ps = pfft.tile([2 * S, 1024], f32, tag="pmm")
                    for q in range(2):
                        cq = half * 2 + q
                        nc.tensor.matmul(ps[:, bass.ts(q, 512)], ct["w2"][:],
                                         t1gv[:, bass.ts(cq, 8), :])
                    nc.vector.tensor_copy(s2[:, bass.ts(half, 1024)], ps[:])
            else:
                for half in range(2):
                    ps = pfft.tile([2 * S, 1024], f32, tag="pmm")
                    for q in range(2):
                        nc.tensor.matmul(ps[:, bass.ts(q, 512)], ct["w2"][:],
                                         t1g[:, bass.ds(half * 1024 + q * 512, 512)])
                    nc.vector.tensor_copy(
                        s2[:].rearrange("p (c v) -> p v c", c=C, v=S)[:, bass.ts(half, 32), :],
                        ps[:].rearrange("p (v c) -> p v c", v=32, c=C))
            nc.sync.dma_start(D2[b][0:S, :], s2[0:S, :])
            nc.scalar.dma_start(D2[b][S:2 * S, 0:(C - 1) * S],
                                s2[S:2 * S, 0:(C - 1) * S])

        def build_dx(b):
            """D2 reads into padded dx layout + reflect pads, split in row
            chunks so conv T0/T1 can start before the full transfer lands."""
            dx2 = dxpool.tile([2 * C2, NPAD], bf16, tag="dx2", name=f"dx2_{b}")
            dxv = dx2[:, 0:NPAD].rearrange("q (a b) -> q a b", a=SP, b=SP)
            d2v = D2[b].rearrange("(ri u) (c v) -> ri c u v", ri=2, u=S, c=C, v=S)
            # row chunks (interior row ranges): [1,34) and [34,65)
            RCH = ((1, 34), (34, S + 1))
            for (lo, hi) in RCH:
                nc.sync.dma_start(dxv[0:32, lo:hi, 1:S + 1], d2v[0][:, lo - 1:hi - 1])
                nc.scalar.dma_start(dxv[32:64, lo:hi, 1:S + 1], d2v[1][:, lo - 1:hi - 1])
                nc.gpsimd.dma_start(dxv[64:96, lo:hi, 0:S], d2v[0][:, lo - 1:hi - 1])
                nc.gpsimd.dma_start(dxv[96:128, lo:hi, 0:S], d2v[1][:, lo - 1:hi - 1])
                q = slice(0, C2)
                nc.vector.tensor_copy(dxv[q, lo:hi, 0:1], dxv[q, lo:hi, 2:3])
                nc.vector.tensor_copy(dxv[q, lo:hi, SP - 1:SP],
                                      dxv[q, lo:hi, SP - 3:SP - 2])
            q = slice(0, C2)
            nc.vector.tensor_copy(dxv[q, 0:1, :], dxv[q, 2:3, :])
            nc.vector.tensor_copy(dxv[q, SP - 1:SP, :], dxv[q, SP - 3:SP - 2, :])
            qb = slice(C2, 2 * C2)
            nc.vector.tensor_copy(dxv[qb, 0:1, 0:S], dxv[qb, 2:3, 0:S])
            nc.vector.tensor_copy(dxv[qb, SP - 1:SP, 0:S],
                                  dxv[qb, SP - 3:SP - 2, 0:S])
            return dx2

        def conv_fc0(b, dx2):
            """3x3 convs + fc0 + LeakyReLU + LN partial sums."""
            dxv = dx2[:, 0:NPAD].rearrange("q (a b) -> q a b", a=SP, b=SP)
            s1c = spool.tile([128, 8], f32, tag="s1cols", name=f"s1c_{b}")
            s2c = spool.tile([128, 4], f32, tag="s2cols", name=f"s2c_{b}")
            s1cols[b], s2cols[b] = s1c, s2c
            for m in range(2):
                h_tiles[(b, m)] = hpool.tile([128, NPIX], bf16, tag="h",
                                             name=f"h_{b}_{m}")
            for T in range(4):
                r0 = T * 16
                psy = pmm.tile([2 * C2, 1024], f32, tag="pmm")
                for q in range(2):
                    rq = r0 + q * 8
                    for di in range(3):
                        nc.tensor.matmul(
                            psy[:, bass.ts(q, 512)],
                            ct["wpair"][:, bass.ts(di, 2 * C2)],
                            dxv[:, rq + di:rq + di + 8, 0:S],
                            start=(di == 0), stop=False)
                    for di in range(2):
                        nc.tensor.matmul(
                            psy[:, bass.ts(q, 512)],
                            ct["wsing"][:, bass.ts(di, 2 * C2)],
                            dxv[0:C2, rq + di:rq + di + 8, 2:SP],
                            start=False, stop=False)
                psh = {m: pmm.tile([128, 1024], f32, tag="pmm",
                                   name=f"psh_{b}_{T}_{m}") for m in range(2)}
                # row-tiled concurrent pairs: single(di=2)@rows0:63 with
                # fc0a-m0@rows64:127 (B-half rhs), then fc0a-m1 q0||q1.
                for q in range(2):
                    rq = r0 + q * 8
                    nc.tensor.matmul(
                        psy[:, bass.ts(q, 512)],
                        ct["wsing"][:, bass.ts(2, 2 * C2)],
                        dxv[0:C2, rq + 2:rq + 10, 2:SP],
                        start=False, stop=True, tile_position=(0, 0))
                    nc.tensor.matmul(
                        psh[0][:, bass.ts(q, 512)],
                        ct["fc0a2"][C2:2 * C2, bass.ts(0, 128)],
                        dxv[C2:2 * C2, rq + 1:rq + 9, 0:S],
                        start=True, stop=False, tile_position=(64, 0))
                nc.tensor.matmul(psh[1][:, bass.ts(0, 512)],
                                 ct["fc0a2"][0:C2, bass.ts(1, 128)],
                                 dxv[0:C2, r0 + 1:r0 + 9, 1:S + 1],
                                 start=True, stop=False, tile_position=(0, 0))
                nc.tensor.matmul(psh[1][:, bass.ts(1, 512)],
                                 ct["fc0a2"][C2:2 * C2, bass.ts(1, 128)],
                                 dxv[C2:2 * C2, r0 + 9:r0 + 17, 0:S],
                                 start=True, stop=False, tile_position=(64, 0))
                yc = ypool.tile([2 * C2, 1024], bf16, tag="yc")
                nc.vector.tensor_copy(yc[:], psy[:])
                for m in range(2):
                    for q in range(2):
                        nc.tensor.matmul(psh[m][:, bass.ts(q, 512)],
                                         ct["fc0bb"][:, bass.ts(m, 128)],
                                         yc[:, bass.ts(q, 512)],
                                         start=False, stop=True)
                    nc.scalar.activation(
                        h_tiles[(b, m)][:, bass.ts(T, 1024)], psh[m][:],
                        AF.Lrelu, bias=ct["fc0b2"][:, m:m + 1], scale=1.0,
                        alpha=0.01, accum_out=s1c[:, T * 2 + m:T * 2 + m + 1])

        def stats_pre(b):
            """sq-pass (TT + reduce) -> per-partition sums -> ones-matmul,
            plus the hw-pass (h *= ln_w), all independent of the pair."""
            scr = scrpool.tile([128, 2048], bf16, tag="sqscr")
            for m in range(2):
                for t in range(2):
                    hs = h_tiles[(b, m)][:, bass.ts(t, 2048)]
                    nc.vector.tensor_tensor(out=scr[:], in0=hs, in1=hs,
                                            op=ALU.mult)
                    nc.vector.tensor_reduce(
                        s2cols[b][:, m * 2 + t:m * 2 + t + 1], scr[:],
                        axis=mybir.AxisListType.X, op=ALU.add)
            stats2 = spool.tile([128, 2], f32, tag="stats2", name=f"stats2_{b}")
            nc.vector.tensor_reduce(stats2[:, 0:1], s1cols[b][:],
                                    axis=mybir.AxisListType.X, op=ALU.add)
            nc.vector.tensor_reduce(stats2[:, 1:2], s2cols[b][:],
                                    axis=mybir.AxisListType.X, op=ALU.add)
            pst = pmm.tile([128, 2], f32, tag="pmm", name=f"pst_{b}")
            nc.tensor.matmul(pst[:], ones[:], stats2[:])
            psts = spool.tile([128, 2], f32, tag="psts", name=f"psts_{b}")
            nc.vector.tensor_copy(psts[:], pst[:])
            stats[b] = {"pst": psts}
            for m in range(2):
                nc.vector.tensor_mul(h_tiles[(b, m)][:], h_tiles[(b, m)][:],
                                     ct["lnw"][:, bass.ts(m, NPIX)])

        def stats_pair(pair):
            """Merged scalar chain for both batches: one Sqrt instruction."""
            b0, b1 = 2 * pair, 2 * pair + 1
            mu2 = spool.tile([128, 2], f32, tag="stat2", name=f"mu2_{pair}")
            ex2 = spool.tile([128, 2], f32, tag="stat2", name=f"ex2_{pair}")
            for j, bb in ((0, b0), (1, b1)):
                nc.vector.tensor_scalar(out=mu2[:, j:j + 1],
                                        in0=stats[bb]["pst"][:, 0:1],
                                        scalar1=1.0 / LN_N, scalar2=None,
                                        op0=ALU.mult)
                nc.vector.tensor_scalar(out=ex2[:, j:j + 1],
                                        in0=stats[bb]["pst"][:, 1:2],
                                        scalar1=1.0 / LN_N, scalar2=None,
                                        op0=ALU.mult)
            msq = spool.tile([128, 2], f32, tag="stat2", name=f"msq_{pair}")
            nc.vector.tensor_tensor(out=msq[:], in0=mu2[:], in1=mu2[:],
                                    op=ALU.mult)
            vpe = spool.tile([128, 2], f32, tag="stat2", name=f"vpe_{pair}")
            nc.vector.tensor_tensor(out=vpe[:], in0=ex2[:], in1=msq[:],
                                    op=ALU.subtract)
            nc.vector.tensor_scalar_add(vpe[:], vpe[:], EPS)
            sd = spool.tile([128, 2], f32, tag="stat2", name=f"sd_{pair}")
            nc.scalar.activation(sd[:], vpe[:], AF.Sqrt, bias=0.0, scale=1.0)
            rr = spool.tile([128, 2], f32, tag="stat2", name=f"rr_{pair}")
            nc.vector.reciprocal(rr[:], sd[:])
            nrm = spool.tile([128, 2], f32, tag="stat2", name=f"nrm_{pair}")
            nc.vector.tensor_tensor(out=nrm[:], in0=rr[:], in1=mu2[:],
                                    op=ALU.mult)
            nc.vector.tensor_scalar_mul(nrm[:], nrm[:], -1.0)
            # pack per-batch halves: rows 0:64 <- b0 col, rows 64:128 <- b1 col
            r2 = spool.tile([128, 1], f32, tag="statp", name=f"r2_{pair}")
            nrm2 = spool.tile([128, 1], f32, tag="statp", name=f"nrm2_{pair}")
            nc.vector.tensor_copy(r2[0:64, :], rr[0:64, 0:1])
            nc.vector.tensor_copy(r2[64:128, :], rr[64:128, 1:2])
            nc.vector.tensor_copy(nrm2[0:64, :], nrm[0:64, 0:1])
            nc.vector.tensor_copy(nrm2[64:128, :], nrm[64:128, 1:2])
            return r2, nrm2

        def fc1_tail(pair):
            b0, b1 = 2 * pair, 2 * pair + 1
            r2, nrm2 = stats_pair(pair)
            z = zpool.tile([128, NPIX], bf16, tag="ztile")
            nc.vector.scalar_tensor_tensor(
                out=z[:], in0=ct["lw1t"][:], scalar=nrm2[:], in1=ct["lbt"][:],
                op0=ALU.mult, op1=ALU.add)
            mask2 = mpool.tile([128, NPIX], bf16, tag="mask2", name=f"mask2_{pair}")
            nc.sync.dma_start(mask2[:], maskd[pair][:])
            dm = dmpool.tile([128, NPIX], bf16, tag="dm")
            for T in range(4):
                ps = pmm.tile([128, 1024], f32, tag="pmm", name=f"psd_{pair}_{T}")
                for q in range(2):
                    for m in range(2):
                        for half, b in ((0, b0), (1, b1)):
                            nc.tensor.matmul(
                                ps[bass.ts(half, 64), bass.ts(q, 512)],
                                ct["fc1"][:, bass.ts(m, 64)],
                                h_tiles[(b, m)][:, bass.ds(T * 1024 + q * 512, 512)],
                                start=(m == 0), stop=(m == 1),
                                tile_position=(0, half * 64))
                nc.vector.scalar_tensor_tensor(
                    out=dm[:, bass.ts(T, 1024)], in0=ps[:],
                    scalar=r2[:], in1=z[:, bass.ts(T, 1024)],
                    op0=ALU.mult, op1=ALU.add)
            nc.vector.tensor_mul(dm[:], dm[:], mask2[:])
            for hb, b in ((0, b0), (1, b1)):
                eng = nc.sync if hb == 0 else nc.scalar
                for ri in range(2):
                    nc.sync.dma_start(
                        D3[pair][hb].rearrange("(ri u) (c v) -> ri c u v",
                                               ri=2, u=S, c=C, v=S)[ri],
                        dm[bass.ds(hb * 64 + ri * 32, 32), :].rearrange(
                            "c (u v) -> c u v", u=S, v=S))
                dg = gpool.tile([2 * S, C * S], bf16, tag="dg", name=f"dg_{b}")
                d3g = D3[pair][hb].rearrange("(ri u) (c v) -> ri u c v",
                                             ri=2, u=S, c=C, v=S)
                for ri in range(2):
                    eng.dma_start(
                        dg[bass.ts(ri, S), :].rearrange("p (c v) -> p c v", c=C, v=S),
                        d3g[ri])
                dgath[b] = dg

        def ifft_out(b):
            upd = dgath[b]
            sa = sapool.tile([2 * S, S * C], bf16, tag="sa", name=f"sa_{b}")
            if STRIDED_RHS:
                updv = upd[:, :].rearrange("p (c v) -> p v c", c=C, v=S)
                for half in range(2):
                    ps = pfft.tile([2 * S, 1024], f32, tag="pmm")
                    for q in range(2):
                        vq = half * 2 + q
                        nc.tensor.matmul(ps[:, bass.ts(q, 512)], ct["wa"][:],
                                         updv[:, bass.ts(vq, 16), :])
                    nc.vector.tensor_copy(sa[:, bass.ts(half, 1024)], ps[:])
            else:
                for half in range(2):
                    ps = pfft.tile([2 * S, 1024], f32, tag="pmm")
                    for q in range(2):
                        nc.tensor.matmul(ps[:, bass.ts(q, 512)], ct["wa"][:],
                                         upd[:, bass.ds(half * 1024 + q * 512, 512)])
                    nc.vector.tensor_copy(
                        sa[:].rearrange("p (v c) -> p c v", v=S, c=C)[:, bass.ts(half, 16), :],
                        ps[:].rearrange("p (c v) -> p c v", c=16, v=S))
            nc.sync.dma_start(D4[b][:], sa[:])
            dgb = gbpool.tile([2 * S, S * C], bf16, tag="dgb", name=f"dgb_{b}")
            d4v = D4[b].rearrange("(ri a) (v c) -> ri v a c", ri=2, a=S, v=S, c=C)
            nc.sync.dma_start(
                dgb[bass.ts(0, S), :].rearrange("p (a c) -> p a c", a=S, c=C), d4v[0])
            nc.scalar.dma_start(
                dgb[bass.ts(1, S), :].rearrange("p (a c) -> p a c", a=S, c=C), d4v[1])
            sb = sbpool.tile([2 * S, S * C], bf16, tag="sb", name=f"sb_{b}")
            for half in range(2):
                ps = pfft.tile([2 * S, 1024], f32, tag="pmm")
                for q in range(2):
                    nc.tensor.matmul(ps[:, bass.ts(q, 512)], ct["wa"][:],
                                     dgb[:, bass.ds(half * 1024 + q * 512, 512)])
                nc.vector.tensor_copy(sb[:, bass.ts(half, 1024)], ps[:])
            nc.sync.dma_start(OUT[b][:], sb[:])

        assert steps == 1, "device program built for steps==1"
        # ---- software-pipelined emission, 2-deep front-end prefetch ----
        # fronts: 0,1 up-front; dx(b) as early as deps allow; tails late.
        fft_front(0)
        fft_front(1)
        dx0 = build_dx(0)
        fft_front(2)
        dx1 = build_dx(1)
        conv_fc0(0, dx0)
        fft_front(3)
        dx2_ = build_dx(2)
        stats_pre(0)
        conv_fc0(1, dx1)
        stats_pre(1)
        dx3 = build_dx(3)
        fc1_tail(0)
        ifft_out(0)
        conv_fc0(2, dx2_)
        stats_pre(2)
        ifft_out(1)
        conv_fc0(3, dx3)
        stats_pre(3)
        fc1_tail(1)
        ifft_out(2)
        ifft_out(3)

    return nc


_BUILT = {}


def kernel(**inputs):
    x = np.ascontiguousarray(np.asarray(inputs["x"], dtype=np.float32))
    steps = int(np.asarray(inputs["steps"]))
    if steps == 0:
        return x.astype(np.complex64)
    assert steps == 1, f"unsupported steps={steps}"

    cst = host_constants(inputs)
    su = np.asarray(inputs["stoch_u"], dtype=np.float32)[..., 0]   # [B, S, S]
    mask = (su > FIRE).astype(np.float32)
    mask_dev = np.ascontiguousarray(np.transpose(mask, (0, 2, 1))
                                    ).reshape(B, NPIX).astype(_BF)
    mask_pairs = np.empty((B // 2, 128, NPIX), _BF)
    for p in range(B // 2):
        mask_pairs[p, :64] = mask_dev[2 * p][None, :]
        mask_pairs[p, 64:] = mask_dev[2 * p + 1][None, :]

    if "nc" not in _BUILT:
        nc = build_nc(steps=1)
        nc.finalize()
        _BUILT["nc"] = nc
    nc = _BUILT["nc"]

    in_maps = []
    for core in range(NCORES):
        m = {k: np.ascontiguousarray(v) for k, v in cst.items()}
        m["xs"] = x[core * BPC:(core + 1) * BPC].astype(_BF)
        m["maskd"] = mask_pairs[core * (BPC // 2):(core + 1) * (BPC // 2)]
        in_maps.append(m)

    from concourse.bass_utils import run_bass_kernel_spmd
    trace = bool(int(os.environ.get("KERNEL_TRACE", "0")))
    res = run_bass_kernel_spmd(nc, in_maps, list(range(NCORES)), trace=trace)
    if trace and res.exec_time_ns is not None:
        print(f"HW exec time: {res.exec_time_ns} ns")
        if res.instructions_and_trace is not None:
            print("trace:", res.instructions_and_trace[1])

    out = np.empty((B, S, S, C), np.complex64)
    for core in range(NCORES):
        o = np.asarray(res.results[core]["OUT"], dtype=np.float32)  # [BPC,128,2048]
        for j in range(BPC):
            b = core * BPC + j
            re = o[j, :S].reshape(S, S, C)
            im = o[j, S:].reshape(S, S, C)
            out[b] = x[b] + re + 1j * im
    return out
